# revision 1
# baseline (speedup 1.0000x reference)
"""MoE (top-k routing + SwiGLU expert MLP) Trainium2 kernel, 8 NeuronCores.

Strategy
--------
Routing-aware expert-parallel sharding. On the host we compute the (tiny)
gating network in float64 (logits -> softmax -> top-k sets + gate values;
selection matches the fp32 reference for any non-degenerate margin), then
dispatch each expert's routed tokens to a pair of cores:

    core c   ->  expert e = c // 2,  intermediate half h = c % 2

Each core runs a dense SwiGLU MLP shard in bf16 on its routed token batch:
    gate = x @ Wg^T, up = x @ Wu^T          (Wg/Wu: this core's 4096 rows
                                             of w_gate_up[e])
    hidden = up * silu(gate)
    y2 = Wd_half^T-partial @ hidden          (contraction over this core's
                                             4096-wide slice of I)
returning an UNSCALED partial expert output y2 [H, M] in fp32. The host sums
the two halves of each expert, applies the gate values, and scatter-adds into
the full [S, H] output. (Summing w-scaled partials over all cores equals the
reference's top-k weighted combine; doing the linear combine host-side avoids
a device all-reduce entirely.)

Device kernel (per core, all in one TileContext, fully unrolled):
    mm1:  out [4096(I'), M]  = W(g|u)T-tiles (stationary) x xT (moving)
    swiglu: ACT silu on gate psum, DVE mul with up psum -> hidden bf16 SBUF
    mm2:  out [H, M] = WdT-tiles (stationary) x hidden (moving), fp32 out
xT is fully SBUF-resident; hidden stays SBUF-resident; weights stream once
(~75 MB bf16 per core) and every weight byte is used exactly once.
"""

import os

import ml_dtypes
import numpy as np

import concourse.bass as bass
import concourse.mybir as mybir
import concourse.tile as tile
from bass_rust import SyncInfo
from concourse.bass_utils import run_bass_kernel_spmd

NCORES = 8
P = 128
BF16 = mybir.dt.bfloat16
F32 = mybir.dt.float32
# Above this token capacity the SBUF-resident xT+hidden no longer fit; the
# host then runs the same kernel over sequential token batches.
MAX_M = 1280


def _split_excess_waits(nc, max_sync=1):
    """walrus in this container rejects >~2 sync commands per instruction
    (CoreV3 setupSyncWait).  Hoist excess sem waits onto NoOps that run
    immediately before the offending instruction on the same engine."""
    for bb in nc.m.functions[0].blocks:
        new, changed = [], False
        for ins in bb.instructions:
            si = ins.sync_info
            if si is None:
                new.append(ins)
                continue
            waits = list(si.on_wait)
            n_upd = len(si.on_update)
            if len(waits) + n_upd > max_sync and len(waits) > 1:
                keep = max(1, max_sync - n_upd)
                extra, kept = waits[: len(waits) - keep], waits[len(waits) - keep :]
                for j in range(0, len(extra), max_sync):
                    nop = mybir.InstNoOp(name=f"{ins.name}_waitsplit_{j}")
                    nop.engine = ins.engine
                    nop.sync_info = SyncInfo(
                        on_wait=extra[j : j + max_sync], on_update=[]
                    )
                    nc.register_instruction(nop)
                    new.append(nop)
                ins.sync_info = SyncInfo(on_wait=kept, on_update=si.on_update)
                changed = True
            new.append(ins)
        if changed:
            bb.instructions = new


def _build_kernel(M, H, ISH):
    """One-core program (SPMD across 8 cores): SwiGLU MLP shard.

    Inputs : xt [H, M] bf16, wg/wu [H, ISH] bf16, wd [ISH, H] bf16
    Output : y2 [H, M] fp32   (partial expert output, transposed)
    """
    KO = H // P      # k-tiles over hidden dim (contraction of mm1)
    IJ = ISH // P    # i-tiles over this core's intermediate slice
    HB = H // P      # output-row tiles of mm2
    # balanced token chunks (all ~equal, 16-aligned) — a tiny remainder chunk
    # would run at the LDWEIGHTS floor instead of the streaming rate
    n_ch = -(-M // 512)
    base = (M // n_ch) // 16 * 16
    sizes = [base] * n_ch
    for i in range(-(-(M - base * n_ch) // 16)):
        sizes[i] += 16
    sizes[-1] = M - sum(sizes[:-1])
    chunks, o = [], 0
    for s in sizes:
        chunks.append((o, s))
        o += s

    nc = bass.Bass("TRN2", num_devices=NCORES)
    xt = nc.dram_tensor("xt", [H, M], BF16, kind="ExternalInput")
    wg = nc.dram_tensor("wg", [H, ISH], BF16, kind="ExternalInput")
    wu = nc.dram_tensor("wu", [H, ISH], BF16, kind="ExternalInput")
    wd = nc.dram_tensor("wd", [ISH, H], BF16, kind="ExternalInput")
    y2 = nc.dram_tensor("y2", [H, M], F32, kind="ExternalOutput")

    # wd strips are 1 MB on a single ~31 GB/s DMA queue vs ~14.5 us of matmul
    # per strip — needs >=3 strips in flight to keep mm2 fed. SBUF only
    # allows the deep prefetch at moderate M.
    wd_bufs = 4 if M <= 1120 else (2 if M < MAX_M else 1)
    with tile.TileContext(nc) as tc:
        with (
            tc.tile_pool(name="xp", bufs=1) as xp,
            tc.tile_pool(name="hp", bufs=1) as hp,
            tc.tile_pool(name="wp", bufs=2) as wp,
            tc.tile_pool(name="wdp", bufs=wd_bufs) as wdp,
            tc.tile_pool(name="sgp", bufs=3) as sgp,
            tc.tile_pool(name="stp", bufs=3) as stp,
            tc.tile_pool(name="psp", bufs=2, space="PSUM") as psp,
        ):
            def load_w(j):
                wgt = wp.tile([P, KO, P], BF16, tag="wg", name=f"wg_{j}")
                nc.sync.dma_start(
                    wgt[:],
                    wg[:, j * P : (j + 1) * P].rearrange("(ko p) i -> p ko i", p=P),
                )
                wut = wp.tile([P, KO, P], BF16, tag="wu", name=f"wu_{j}")
                nc.sync.dma_start(
                    wut[:],
                    wu[:, j * P : (j + 1) * P].rearrange("(ko p) i -> p ko i", p=P),
                )
                return wgt, wut

            # j0 weights are issued BEFORE x so the k-outer first block below
            # can consume x tiles as they stream in
            w0 = load_w(0)

            # one tile per ko so matmuls only wait on the x rows they read
            x_sb = [
                xp.tile([P, M], BF16, tag=f"x{ko}", name=f"x{ko}")
                for ko in range(KO)
            ]
            for ko in range(KO):
                nc.sync.dma_start(x_sb[ko][:], xt[ko * P : (ko + 1) * P, :])

            hid = hp.tile([P, IJ, M], BF16)

            def swiglu(j, pg, pu, off, sz):
                sg = sgp.tile([P, 512], F32, tag="sg", name=f"sg_{j}")
                nc.scalar.activation(
                    sg[:, :sz], pg[:, :sz], mybir.ActivationFunctionType.Silu
                )
                nc.vector.tensor_mul(hid[:, j, off : off + sz], sg[:, :sz], pu[:, :sz])

            # ---- mm1 + SwiGLU: hidden[i, m] = up * silu(gate) ----
            for j in range(IJ):
                if j == 0:
                    wgt, wut = w0
                    # k-outer over all chunk psum groups: each x k-tile is
                    # consumed the moment its DMA lands, so the PE works (and
                    # HAM warms) through the initial x load instead of
                    # stalling until the last tile arrives
                    pgs = [
                        psp.tile([P, 512], F32, tag="pg", bufs=len(chunks), name=f"pg0_{ci}")
                        for ci in range(len(chunks))
                    ]
                    pus = [
                        psp.tile([P, 512], F32, tag="pu", bufs=len(chunks), name=f"pu0_{ci}")
                        for ci in range(len(chunks))
                    ]
                    for k in range(KO):
                        for ci, (off, sz) in enumerate(chunks):
                            nc.tensor.matmul(
                                pgs[ci][:, :sz],
                                wgt[:, k, :],
                                x_sb[k][:, off : off + sz],
                                start=(k == 0),
                                stop=(k == KO - 1),
                            )
                        for ci, (off, sz) in enumerate(chunks):
                            nc.tensor.matmul(
                                pus[ci][:, :sz],
                                wut[:, k, :],
                                x_sb[k][:, off : off + sz],
                                start=(k == 0),
                                stop=(k == KO - 1),
                            )
                    for ci, (off, sz) in enumerate(chunks):
                        swiglu(0, pgs[ci], pus[ci], off, sz)
                    continue
                wgt, wut = load_w(j)
                for ci, (off, sz) in enumerate(chunks):
                    pg = psp.tile([P, 512], F32, tag="pg", bufs=len(chunks))
                    for k in range(KO):
                        nc.tensor.matmul(
                            pg[:, :sz],
                            wgt[:, k, :],
                            x_sb[k][:, off : off + sz],
                            start=(k == 0),
                            stop=(k == KO - 1),
                        )
                    pu = psp.tile([P, 512], F32, tag="pu", bufs=len(chunks))
                    for k in range(KO):
                        nc.tensor.matmul(
                            pu[:, :sz],
                            wut[:, k, :],
                            x_sb[k][:, off : off + sz],
                            start=(k == 0),
                            stop=(k == KO - 1),
                        )
                    swiglu(j, pg, pu, off, sz)

            # ---- mm2: y2[h, m] = sum_i wd[i, h] * hidden[i, m] ----
            for hb in range(HB):
                wdt = wdp.tile([P, IJ, P], BF16, tag="wd")
                nc.sync.dma_start(
                    wdt[:],
                    wd[:, hb * P : (hb + 1) * P].rearrange("(j p) h -> p j h", p=P),
                )
                for off, sz in chunks:
                    po = psp.tile([P, 512], F32, tag="pg", bufs=len(chunks), name=f"po_{hb}")
                    for j in range(IJ):
                        nc.tensor.matmul(
                            po[:, :sz],
                            wdt[:, j, :],
                            hid[:, j, off : off + sz],
                            start=(j == 0),
                            stop=(j == IJ - 1),
                        )
                    ot = stp.tile([P, 512], F32)
                    nc.vector.tensor_copy(ot[:, :sz], po[:, :sz])
                    nc.sync.dma_start(
                        y2[hb * P : (hb + 1) * P, off : off + sz], ot[:, :sz]
                    )

    _split_excess_waits(nc)
    return nc


def _route(x2d, gate_w, k):
    """Host gating in float64: top-k sets + gate values per token."""
    logits = x2d.astype(np.float64) @ gate_w.astype(np.float64).T
    logits -= logits.max(axis=-1, keepdims=True)
    p = np.exp(logits)
    p /= p.sum(axis=-1, keepdims=True)
    topk = np.argsort(-p, axis=-1, kind="stable")[:, :k]  # [S, k]
    return p, topk


def kernel(x, gate_w, w_gate_up, w_down, top_k):
    kernel.last_exec_time_ns = None
    x = np.asarray(x)
    gate_w = np.asarray(gate_w)
    w_gate_up = np.asarray(w_gate_up)
    w_down = np.asarray(w_down)
    k = int(np.asarray(top_k))

    B, S, H = x.shape
    E = gate_w.shape[0]
    I = w_down.shape[2]
    ISH = I // (NCORES // E)  # per-core slice of the intermediate dim
    x2d = x.reshape(-1, H)
    n_tok = x2d.shape[0]

    p, topk = _route(x2d, gate_w, k)
    sel = [np.nonzero((topk == e).any(axis=-1))[0] for e in range(E)]
    counts = [len(s) for s in sel]
    max_count = max(max(counts), 1)

    # token batching if an expert's load exceeds the single-pass capacity
    n_batches = -(-max_count // MAX_M)
    per_batch = -(-max_count // n_batches)
    # 4-token alignment is load-bearing: M=1049 (odd chunk widths) measured
    # +206 us — odd-width transfers/ops fall off a fast path
    M = max(-(-per_batch // 4) * 4, 128)

    bf = ml_dtypes.bfloat16
    # per-core weight shards (host transpose + bf16 cast)
    w_in = []
    for c in range(NCORES):
        e, h = c // 2, c % 2
        wg_s = w_gate_up[e, h * ISH : (h + 1) * ISH, :]          # [ISH, H]
        wu_s = w_gate_up[e, I + h * ISH : I + (h + 1) * ISH, :]  # [ISH, H]
        wd_s = w_down[e][:, h * ISH : (h + 1) * ISH]             # [H, ISH]
        w_in.append(
            {
                "wg": np.ascontiguousarray(wg_s.T).astype(bf),
                "wu": np.ascontiguousarray(wu_s.T).astype(bf),
                "wd": np.ascontiguousarray(wd_s.T).astype(bf),
            }
        )

    nc = _build_kernel(M, H, ISH)
    trace = bool(int(os.environ.get("BASS_TRACE", "0") or "0"))

    y = np.zeros((n_tok, H), dtype=np.float32)
    exec_times = []
    for b in range(n_batches):
        in_maps = []
        for c in range(NCORES):
            e = c // 2
            idx = sel[e][b * M : (b + 1) * M]
            xt = np.zeros((H, M), dtype=bf)
            if len(idx):
                xt[:, : len(idx)] = x2d[idx].T.astype(bf)
            in_maps.append({"xt": xt, **w_in[c]})
        try:
            res = run_bass_kernel_spmd(
                nc, in_maps, core_ids=list(range(NCORES)), trace=trace
            )
        except Exception:
            # transient device/profiling hiccups: one untraced retry
            os.environ["BASS_NEVER_TRACE"] = "1"
            try:
                res = run_bass_kernel_spmd(
                    nc, in_maps, core_ids=list(range(NCORES)), trace=False
                )
            finally:
                os.environ.pop("BASS_NEVER_TRACE", None)
        if res.exec_time_ns is not None:
            exec_times.append(res.exec_time_ns)
        for e in range(E):
            idx = sel[e][b * M : (b + 1) * M]
            if len(idx) == 0:
                continue
            part = (
                res.results[2 * e]["y2"][:, : len(idx)]
                + res.results[2 * e + 1]["y2"][:, : len(idx)]
            ).T  # [n_idx, H] fp32
            y[idx] += p[idx, e].astype(np.float32)[:, None] * part

    if exec_times:
        kernel.last_exec_time_ns = max(exec_times)
    return y.reshape(B, S, H).astype(np.float32)


kernel.last_exec_time_ns = None



# revision 2
# speedup vs baseline: 1.0019x; 1.0019x over previous
"""MoE (top-k routing + SwiGLU expert MLP) Trainium2 kernel, 8 NeuronCores.

Strategy (v2: routing-balanced intermediate-slice sharding)
-----------------------------------------------------------
Host computes the (tiny) gating network in float64 (logits -> softmax ->
top-k sets + gate values). Every core owns the SAME I/8 slice position of
ALL four experts' weights:

    core c  ->  intermediate columns [c*1024, (c+1)*1024) of every expert

and the kernel runs 4 sequential expert phases; in phase e every core
processes expert e's full routed token batch against its I/8 weight slice.
Per-core PE work is therefore sum_e count_e (perfectly balanced regardless
of routing imbalance), instead of 2.25 * max_e count_e for the 2-cores-
per-expert layout.

Per phase (per core, bf16 matmuls, fp32 PSUM):
    mm1:  gate/up [1024(I'), M] = W(g|u)T-tiles (stationary) x xT (moving)
    swiglu: ACT silu on gate psum, DVE mul with up psum -> hidden bf16 SBUF
    mm2:  y2 [H, M] partial = WdT-tiles (stationary) x hidden (moving)

Weights are host-pre-tiled so every weight tile is ONE fully contiguous
DRAM read (786KB / 262KB); weight loads stream on the SP HWDGE ring while
x loads + y2 stores ride the ACT HWDGE ring, so input streams never queue
behind output stores. The host sums the 8 per-core partials (over the I
slices), applies the top-k gate values, and scatter-adds into [S, H].
"""

import os
import sys
import types

import ml_dtypes
import numpy as np

import concourse.bass as bass
import concourse.mybir as mybir
import concourse.tile as tile
from bass_rust import SyncInfo
from concourse.bass_utils import run_bass_kernel_spmd

NCORES = 8
P = 128
BF16 = mybir.dt.bfloat16
F32 = mybir.dt.float32


def _ensure_ntff_hook():
    """Best-effort: register the axon NTFF profile hook if the environment's
    ``antenv`` stub lacks ``axon_hooks`` (otherwise trace=True silently
    degrades and no HW exec time is reported)."""
    try:
        import antenv  # noqa: F401

        try:
            from antenv.axon_hooks import get_axon_ntff_profile_hook
        except ImportError:
            mod = types.ModuleType("antenv.axon_hooks")
            mod._HOOK = None

            def set_axon_ntff_profile_hook(hook, _m=mod):
                _m._HOOK = hook

            def get_axon_ntff_profile_hook(_m=mod):
                return _m._HOOK

            mod.set_axon_ntff_profile_hook = set_axon_ntff_profile_hook
            mod.get_axon_ntff_profile_hook = get_axon_ntff_profile_hook
            sys.modules["antenv.axon_hooks"] = mod
            import antenv as _a

            _a.axon_hooks = mod
        if get_axon_ntff_profile_hook() is None:
            from trn_agent_boot.trn_boot import _ntff_profile_via_ctypes

            from antenv.axon_hooks import set_axon_ntff_profile_hook

            hook = _ntff_profile_via_ctypes("/opt/axon/libaxon_pjrt.so")
            if hook is not None:
                set_axon_ntff_profile_hook(hook)
    except Exception:
        pass


_ensure_ntff_hook()


def _split_excess_waits(nc, max_sync=1):
    """walrus in this container rejects >~2 sync commands per instruction
    (CoreV3 setupSyncWait).  Hoist excess sem waits onto NoOps that run
    immediately before the offending instruction on the same engine."""
    for bb in nc.m.functions[0].blocks:
        new, changed = [], False
        for ins in bb.instructions:
            si = ins.sync_info
            if si is None:
                new.append(ins)
                continue
            waits = list(si.on_wait)
            n_upd = len(si.on_update)
            if len(waits) + n_upd > max_sync and len(waits) > 1:
                keep = max(1, max_sync - n_upd)
                extra, kept = waits[: len(waits) - keep], waits[len(waits) - keep :]
                for j in range(0, len(extra), max_sync):
                    nop = mybir.InstNoOp(name=f"{ins.name}_waitsplit_{j}")
                    nop.engine = ins.engine
                    nop.sync_info = SyncInfo(
                        on_wait=extra[j : j + max_sync], on_update=[]
                    )
                    nc.register_instruction(nop)
                    new.append(nop)
                ins.sync_info = SyncInfo(on_wait=kept, on_update=si.on_update)
                changed = True
            new.append(ins)
        if changed:
            bb.instructions = new


def _chunks_of(Mp):
    """Balanced output-column chunks <=512 (PSUM bank is 512 f32), 16-aligned.

    Balanced (not [512..., remainder]) is load-bearing: with ldw-opt disabled
    every matmul re-issues a ~107ns LDWEIGHTS, which only stays hidden when
    the preceding matmul streams >~128 columns — a tiny remainder chunk would
    stall the PE on every (j, k) pass."""
    n_ch = -(-Mp // 512)
    base = (Mp // n_ch) // 16 * 16
    sizes = [base] * n_ch
    for i in range(-(-(Mp - base * n_ch) // 16)):
        sizes[i] += 16
    sizes[-1] = Mp - sum(sizes[:-1])
    out, o = [], 0
    for s in sizes:
        out.append((o, s))
        o += s
    return out


def _build_kernel(phases, H, ISH, E):
    """One-core program (SPMD x8): 4 expert phases of a SwiGLU MLP shard.

    Inputs : xt  [H, Mtot] bf16           (token batches, phase-contiguous)
             wgu [E*2*IJ*P, KO, P] bf16   (pre-tiled gate/up weight tiles)
             wd  [E*HB*P, IJ, P] bf16     (pre-tiled down-proj weight tiles)
    Output : y2  [H, Mtot] fp32           (partial over this core's I-slice)
    """
    KO = H // P          # contraction tiles of mm1 (hidden dim)
    IJ = ISH // P        # i-tiles of this core's intermediate slice
    HB = H // P          # output-row tiles of mm2
    Mtot = sum(Mp for _, Mp in phases)
    Mmax = max(Mp for _, Mp in phases)

    nc = bass.Bass("TRN2", num_devices=NCORES)
    xt = nc.dram_tensor("xt", [H, Mtot], BF16, kind="ExternalInput")
    wgu = nc.dram_tensor("wgu", [E * 2 * IJ * P, KO, P], BF16, kind="ExternalInput")
    wd = nc.dram_tensor("wd", [E * HB * P, IJ, P], BF16, kind="ExternalInput")
    y2 = nc.dram_tensor("y2", [H, Mtot], F32, kind="ExternalOutput")

    with tile.TileContext(nc) as tc:
        with (
            tc.tile_pool(name="xp", bufs=1) as xp,
            tc.tile_pool(name="hp", bufs=1) as hp,
            tc.tile_pool(name="wp", bufs=2) as wp,
            tc.tile_pool(name="wdp", bufs=8) as wdp,
            tc.tile_pool(name="sgp", bufs=3) as sgp,
            tc.tile_pool(name="stp", bufs=4) as stp,
            tc.tile_pool(name="psp", bufs=4, space="PSUM") as psp,
        ):
            # x k-tiles: one tile per ko — tile dependencies are count-based
            # per tile, so separate tiles let each matmul wait only on the
            # k-slab it reads (progressive consumption during the ramp)
            x_sb = [
                xp.tile([P, Mmax], BF16, tag="x", bufs=KO, name=f"x{ko}")
                for ko in range(KO)
            ]
            hid = hp.tile([P, IJ, Mmax], BF16)

            def load_x(ko, off, Mp, engine):
                engine.dma_start(
                    x_sb[ko][:, :Mp], xt[ko * P : (ko + 1) * P, off : off + Mp]
                )

            # phase-0 tokens alternate across both HWDGE rings: a single ring
            # sustains well under the ~300GB/s the first j-pass consumes
            for ko in range(KO):
                load_x(ko, phases[0][0], phases[0][1], nc.scalar if ko % 2 == 0 else nc.sync)

            def load_w(e, j):
                # wg on the SP ring, wu on the ACT ring: per-ring bandwidth
                # (~170-200GB/s) is the scarce resource during the ramp
                wg_t = wp.tile([P, KO, P], BF16, tag="wg", name=f"wg_{e}_{j}")
                gi = ((e * 2 + 0) * IJ + j) * P
                nc.sync.dma_start(wg_t[:], wgu[gi : gi + P])
                wu_t = wp.tile([P, KO, P], BF16, tag="wu", name=f"wu_{e}_{j}")
                ui = ((e * 2 + 1) * IJ + j) * P
                nc.scalar.dma_start(wu_t[:], wgu[ui : ui + P])
                return wg_t, wu_t

            w_pre = {}
            for e, (off, Mp) in enumerate(phases):
                chunks = _chunks_of(Mp)
                # ---- mm1 + SwiGLU ----
                for j in range(IJ):
                    wg_t, wu_t = w_pre.pop((e, j), None) or load_w(e, j)
                    pgs = [
                        psp.tile([P, 512], F32, tag="pg", bufs=4, name=f"pg_{e}_{j}_{ci}")
                        for ci in range(len(chunks))
                    ]
                    pus = [
                        psp.tile([P, 512], F32, tag="pu", bufs=4, name=f"pu_{e}_{j}_{ci}")
                        for ci in range(len(chunks))
                    ]
                    # k-outer: each stationary loaded once per (j, k); x tiles
                    # consumed in DMA arrival order (phase-0 ramp overlap)
                    for k in range(KO):
                        for ci, (o, sz) in enumerate(chunks):
                            nc.tensor.matmul(
                                pgs[ci][:, :sz],
                                wg_t[:, k, :],
                                x_sb[k][:, o : o + sz],
                                start=(k == 0),
                                stop=(k == KO - 1),
                            )
                        for ci, (o, sz) in enumerate(chunks):
                            nc.tensor.matmul(
                                pus[ci][:, :sz],
                                wu_t[:, k, :],
                                x_sb[k][:, o : o + sz],
                                start=(k == 0),
                                stop=(k == KO - 1),
                            )
                    for ci, (o, sz) in enumerate(chunks):
                        sg = sgp.tile([P, 512], F32, tag="sg", name=f"sg_{e}_{j}_{ci}")
                        nc.scalar.activation(
                            sg[:, :sz],
                            pgs[ci][:, :sz],
                            mybir.ActivationFunctionType.Silu,
                        )
                        nc.vector.tensor_mul(
                            hid[:, j, o : o + sz], sg[:, :sz], pus[ci][:, :sz]
                        )

                # hoist the next phase's first weight pairs to BEFORE any y2
                # stores are issued: the wu stream rides the ACT ring, and a
                # wu DMA issued inside the next phase would queue behind all
                # of this phase's stores (HWDGE rings are FIFO), stalling the
                # PE ~3us at the phase boundary
                if e + 1 < len(phases):
                    for j in range(2):
                        w_pre[(e + 1, j)] = load_w(e + 1, j)

                # ---- mm2: y2[h, m] += wd[i, h] * hidden[i, m] ----
                for hb in range(HB):
                    wd_t = wdp.tile([P, IJ, P], BF16, tag="wd", name=f"wd_{e}_{hb}")
                    di = (e * HB + hb) * P
                    nc.sync.dma_start(wd_t[:], wd[di : di + P])
                    # next phase's tokens, interleaved on the sync ring: their
                    # WAR on the x tiles cleared when this phase's mm1 ended,
                    # and the ACT ring stays stores-only (no head-of-line
                    # blocking of y2 stores behind 6MB of x transfers)
                    if e + 1 < len(phases):
                        noff, nMp = phases[e + 1]
                        for ko in range(3 * hb, min(3 * hb + 3, KO)):
                            load_x(ko, noff, nMp, nc.sync)
                    for ci, (o, sz) in enumerate(chunks):
                        po = psp.tile([P, 512], F32, tag="pg", bufs=4, name=f"po_{e}_{hb}_{ci}")
                        for j in range(IJ):
                            nc.tensor.matmul(
                                po[:, :sz],
                                wd_t[:, j, :],
                                hid[:, j, o : o + sz],
                                start=(j == 0),
                                stop=(j == IJ - 1),
                            )
                        ot = stp.tile([P, 512], F32, tag="ot", name=f"ot_{e}_{hb}_{ci}")
                        nc.vector.tensor_copy(ot[:, :sz], po[:, :sz])
                        nc.scalar.dma_start(
                            y2[hb * P : (hb + 1) * P, off + o : off + o + sz],
                            ot[:, :sz],
                        )



    _split_excess_waits(nc)
    return nc


def _route(x2d, gate_w, k):
    """Host gating in float64: top-k sets + gate values per token."""
    logits = x2d.astype(np.float64) @ gate_w.astype(np.float64).T
    logits -= logits.max(axis=-1, keepdims=True)
    p = np.exp(logits)
    p /= p.sum(axis=-1, keepdims=True)
    topk = np.argsort(-p, axis=-1, kind="stable")[:, :k]  # [S, k]
    return p, topk


def kernel(x, gate_w, w_gate_up, w_down, top_k):
    kernel.last_exec_time_ns = None
    x = np.asarray(x)
    gate_w = np.asarray(gate_w)
    w_gate_up = np.asarray(w_gate_up)
    w_down = np.asarray(w_down)
    k = int(np.asarray(top_k))

    B, S, H = x.shape
    E = gate_w.shape[0]
    I = w_down.shape[2]
    ISH = I // NCORES    # per-core I-slice (same slice position, all experts)
    IJ = ISH // P
    KO = H // P
    HB = H // P
    x2d = x.reshape(-1, H)
    n_tok = x2d.shape[0]

    p, topk = _route(x2d, gate_w, k)
    sel = [np.nonzero((topk == e).any(axis=-1))[0] for e in range(E)]
    counts = [len(s) for s in sel]
    # 8-token phase alignment: 16B-aligned bf16 x rows, 32B-aligned f32 y rows
    Mps = [max(16, -(-c // 8) * 8) for c in counts]
    offs = [0]
    for m in Mps[:-1]:
        offs.append(offs[-1] + m)
    Mtot = sum(Mps)
    phases = list(zip(offs, Mps))

    bf = ml_dtypes.bfloat16

    # token batches, phase-contiguous, zero-padded; replicated to all cores
    xt = np.zeros((H, Mtot), dtype=bf)
    for e in range(E):
        if counts[e]:
            xt[:, offs[e] : offs[e] + counts[e]] = x2d[sel[e]].T.astype(bf)

    # pre-tiled weights: every (expert, tile) is one contiguous DRAM block
    # wgu rows: part*I + c*ISH + j*P + ii ; cols: ko*P + p
    t = w_gate_up.astype(bf).reshape(E, 2, NCORES, IJ, P, KO, P)
    t = np.ascontiguousarray(t.transpose(2, 0, 1, 3, 6, 5, 4))
    wgu_t = t.reshape(NCORES, E * 2 * IJ * P, KO, P)

    t = w_down.astype(bf).reshape(E, HB, P, NCORES, IJ, P)
    t = np.ascontiguousarray(t.transpose(3, 0, 1, 5, 4, 2))
    wd_t = t.reshape(NCORES, E * HB * P, IJ, P)

    nc = _build_kernel(phases, H, ISH, E)
    trace = bool(int(os.environ.get("BASS_TRACE", "0") or "0"))

    in_maps = [
        {"xt": xt, "wgu": wgu_t[c], "wd": wd_t[c]} for c in range(NCORES)
    ]
    try:
        res = run_bass_kernel_spmd(
            nc, in_maps, core_ids=list(range(NCORES)), trace=trace
        )
    except Exception:
        # transient device/profiling hiccups: one untraced retry
        os.environ["BASS_NEVER_TRACE"] = "1"
        try:
            res = run_bass_kernel_spmd(
                nc, in_maps, core_ids=list(range(NCORES)), trace=False
            )
        finally:
            os.environ.pop("BASS_NEVER_TRACE", None)
    if res.exec_time_ns is not None:
        kernel.last_exec_time_ns = res.exec_time_ns

    # host combine: sum the 8 I-slice partials, apply gate values, scatter-add
    Y = res.results[0]["y2"].copy()
    for c in range(1, NCORES):
        Y += res.results[c]["y2"]
    y = np.zeros((n_tok, H), dtype=np.float32)
    for e in range(E):
        idx = sel[e]
        if len(idx) == 0:
            continue
        y[idx] += p[idx, e].astype(np.float32)[:, None] * Y[
            :, offs[e] : offs[e] + len(idx)
        ].T
    return y.reshape(B, S, H).astype(np.float32)


kernel.last_exec_time_ns = None


# revision 3
# speedup vs baseline: 1.0065x; 1.0046x over previous
"""MoE (top-k routing + SwiGLU expert MLP) Trainium2 kernel, 8 NeuronCores.

Strategy (v2: routing-balanced intermediate-slice sharding)
-----------------------------------------------------------
Host computes the (tiny) gating network in float64 (logits -> softmax ->
top-k sets + gate values). Every core owns the SAME I/8 slice position of
ALL four experts' weights:

    core c  ->  intermediate columns [c*1024, (c+1)*1024) of every expert

and the kernel runs 4 sequential expert phases; in phase e every core
processes expert e's full routed token batch against its I/8 weight slice.
Per-core PE work is therefore sum_e count_e (perfectly balanced regardless
of routing imbalance), instead of 2.25 * max_e count_e for the 2-cores-
per-expert layout.

Per phase (per core, bf16 matmuls, fp32 PSUM):
    mm1:  gate/up [1024(I'), M] = W(g|u)T-tiles (stationary) x xT (moving)
    swiglu: ACT silu on gate psum, DVE mul with up psum -> hidden bf16 SBUF
    mm2:  y2 [H, M] partial = WdT-tiles (stationary) x hidden (moving)

Weights are host-pre-tiled so every weight tile is ONE fully contiguous
DRAM read (786KB / 262KB); weight loads stream on the SP HWDGE ring while
x loads + y2 stores ride the ACT HWDGE ring, so input streams never queue
behind output stores. The host sums the 8 per-core partials (over the I
slices), applies the top-k gate values, and scatter-adds into [S, H].
"""

import os
import sys
import types

import ml_dtypes
import numpy as np

import concourse.bass as bass
import concourse.mybir as mybir
import concourse.tile as tile
from bass_rust import SyncInfo
from concourse.bass_utils import run_bass_kernel_spmd

NCORES = 8
P = 128
BF16 = mybir.dt.bfloat16
F32 = mybir.dt.float32


def _ensure_ntff_hook():
    """Best-effort: register the axon NTFF profile hook if the environment's
    ``antenv`` stub lacks ``axon_hooks`` (otherwise trace=True silently
    degrades and no HW exec time is reported)."""
    try:
        import antenv  # noqa: F401

        try:
            from antenv.axon_hooks import get_axon_ntff_profile_hook
        except ImportError:
            mod = types.ModuleType("antenv.axon_hooks")
            mod._HOOK = None

            def set_axon_ntff_profile_hook(hook, _m=mod):
                _m._HOOK = hook

            def get_axon_ntff_profile_hook(_m=mod):
                return _m._HOOK

            mod.set_axon_ntff_profile_hook = set_axon_ntff_profile_hook
            mod.get_axon_ntff_profile_hook = get_axon_ntff_profile_hook
            sys.modules["antenv.axon_hooks"] = mod
            import antenv as _a

            _a.axon_hooks = mod
        if get_axon_ntff_profile_hook() is None:
            from trn_agent_boot.trn_boot import _ntff_profile_via_ctypes

            from antenv.axon_hooks import set_axon_ntff_profile_hook

            hook = _ntff_profile_via_ctypes("/opt/axon/libaxon_pjrt.so")
            if hook is not None:
                set_axon_ntff_profile_hook(hook)
    except Exception:
        pass


_ensure_ntff_hook()


def _split_excess_waits(nc, max_sync=1):
    """walrus in this container rejects >~2 sync commands per instruction
    (CoreV3 setupSyncWait).  Hoist excess sem waits onto NoOps that run
    immediately before the offending instruction on the same engine."""
    for bb in nc.m.functions[0].blocks:
        new, changed = [], False
        for ins in bb.instructions:
            si = ins.sync_info
            if si is None:
                new.append(ins)
                continue
            waits = list(si.on_wait)
            n_upd = len(si.on_update)
            if len(waits) + n_upd > max_sync and len(waits) > 1:
                keep = max(1, max_sync - n_upd)
                extra, kept = waits[: len(waits) - keep], waits[len(waits) - keep :]
                for j in range(0, len(extra), max_sync):
                    nop = mybir.InstNoOp(name=f"{ins.name}_waitsplit_{j}")
                    nop.engine = ins.engine
                    nop.sync_info = SyncInfo(
                        on_wait=extra[j : j + max_sync], on_update=[]
                    )
                    nc.register_instruction(nop)
                    new.append(nop)
                ins.sync_info = SyncInfo(on_wait=kept, on_update=si.on_update)
                changed = True
            new.append(ins)
        if changed:
            bb.instructions = new


def _chunks_of(Mp):
    """Balanced output-column chunks <=512 (PSUM bank is 512 f32), 16-aligned.

    Balanced (not [512..., remainder]) is load-bearing: with ldw-opt disabled
    every matmul re-issues a ~107ns LDWEIGHTS, which only stays hidden when
    the preceding matmul streams >~128 columns — a tiny remainder chunk would
    stall the PE on every (j, k) pass."""
    n_ch = -(-Mp // 512)
    base = (Mp // n_ch) // 16 * 16
    sizes = [base] * n_ch
    for i in range(-(-(Mp - base * n_ch) // 16)):
        sizes[i] += 16
    sizes[-1] = Mp - sum(sizes[:-1])
    out, o = [], 0
    for s in sizes:
        out.append((o, s))
        o += s
    return out


def _build_kernel(phases, H, ISH, E):
    """One-core program (SPMD x8): 4 expert phases of a SwiGLU MLP shard.

    Inputs : xt  [H, Mtot] bf16           (token batches, phase-contiguous)
             wgu [E*2*IJ*P, KO, P] bf16   (pre-tiled gate/up weight tiles)
             wd  [E*HB*P, IJ, P] bf16     (pre-tiled down-proj weight tiles)
    Output : y2  [H, Mtot] fp32           (partial over this core's I-slice)
    """
    KO = H // P          # contraction tiles of mm1 (hidden dim)
    IJ = ISH // P        # i-tiles of this core's intermediate slice
    HB = H // P          # output-row tiles of mm2
    Mtot = sum(Mp for _, Mp in phases)
    Mmax = max(Mp for _, Mp in phases)

    nc = bass.Bass("TRN2", num_devices=NCORES)
    xt = nc.dram_tensor("xt", [H, Mtot], BF16, kind="ExternalInput")
    wgu = nc.dram_tensor("wgu", [E * 2 * IJ * P, KO, P], BF16, kind="ExternalInput")
    wd = nc.dram_tensor("wd", [E * HB * P, IJ, P], BF16, kind="ExternalInput")
    y2 = nc.dram_tensor("y2", [H, Mtot], F32, kind="ExternalOutput")

    with tile.TileContext(nc) as tc:
        with (
            tc.tile_pool(name="xp", bufs=1) as xp,
            tc.tile_pool(name="hp", bufs=1) as hp,
            tc.tile_pool(name="wp", bufs=2) as wp,
            tc.tile_pool(name="wdp", bufs=8) as wdp,
            tc.tile_pool(name="sgp", bufs=3) as sgp,
            tc.tile_pool(name="stp", bufs=4) as stp,
            tc.tile_pool(name="psp", bufs=4, space="PSUM") as psp,
        ):
            # x k-tiles: one tile per ko — tile dependencies are count-based
            # per tile, so separate tiles let each matmul wait only on the
            # k-slab it reads (progressive consumption during the ramp)
            x_sb = [
                xp.tile([P, Mmax], BF16, tag="x", bufs=KO, name=f"x{ko}")
                for ko in range(KO)
            ]
            hid = hp.tile([P, IJ, Mmax], BF16)

            def load_x(ko, off, Mp, engine):
                engine.dma_start(
                    x_sb[ko][:, :Mp], xt[ko * P : (ko + 1) * P, off : off + Mp]
                )

            def load_w(e, j):
                # wg on the SP ring, wu on the ACT ring: per-ring bandwidth
                # (~170-200GB/s) is the scarce resource during the ramp
                wg_t = wp.tile([P, KO, P], BF16, tag="wg", name=f"wg_{e}_{j}")
                gi = ((e * 2 + 0) * IJ + j) * P
                nc.sync.dma_start(wg_t[:], wgu[gi : gi + P])
                wu_t = wp.tile([P, KO, P], BF16, tag="wu", name=f"wu_{e}_{j}")
                ui = ((e * 2 + 1) * IJ + j) * P
                nc.scalar.dma_start(wu_t[:], wgu[ui : ui + P])
                return wg_t, wu_t

            # phase-0 j0/j1 weights BEFORE the x burst (rings are FIFO: the
            # first matmul must not queue behind ~3MB of x per ring), then
            # phase-0 tokens alternating across both HWDGE rings: a single
            # ring sustains well under the ~300GB/s the first j-pass consumes
            w_pre = {(0, j): load_w(0, j) for j in range(2)}
            for ko in range(KO):
                load_x(ko, phases[0][0], phases[0][1], nc.scalar if ko % 2 == 0 else nc.sync)

            for e, (off, Mp) in enumerate(phases):
                chunks = _chunks_of(Mp)
                # ---- mm1 + SwiGLU ----
                for j in range(IJ):
                    wg_t, wu_t = w_pre.pop((e, j), None) or load_w(e, j)
                    pgs = [
                        psp.tile([P, 512], F32, tag="pg", bufs=4, name=f"pg_{e}_{j}_{ci}")
                        for ci in range(len(chunks))
                    ]
                    pus = [
                        psp.tile([P, 512], F32, tag="pu", bufs=4, name=f"pu_{e}_{j}_{ci}")
                        for ci in range(len(chunks))
                    ]
                    # k-outer: each stationary loaded once per (j, k); x tiles
                    # consumed in DMA arrival order (phase-0 ramp overlap)
                    for k in range(KO):
                        for ci, (o, sz) in enumerate(chunks):
                            nc.tensor.matmul(
                                pgs[ci][:, :sz],
                                wg_t[:, k, :],
                                x_sb[k][:, o : o + sz],
                                start=(k == 0),
                                stop=(k == KO - 1),
                            )
                        for ci, (o, sz) in enumerate(chunks):
                            nc.tensor.matmul(
                                pus[ci][:, :sz],
                                wu_t[:, k, :],
                                x_sb[k][:, o : o + sz],
                                start=(k == 0),
                                stop=(k == KO - 1),
                            )
                    for ci, (o, sz) in enumerate(chunks):
                        sg = sgp.tile([P, 512], F32, tag="sg", name=f"sg_{e}_{j}_{ci}")
                        nc.scalar.activation(
                            sg[:, :sz],
                            pgs[ci][:, :sz],
                            mybir.ActivationFunctionType.Silu,
                        )
                        nc.vector.tensor_mul(
                            hid[:, j, o : o + sz], sg[:, :sz], pus[ci][:, :sz]
                        )

                # hoist the next phase's first weight pairs to BEFORE any y2
                # stores are issued: the wu stream rides the ACT ring, and a
                # wu DMA issued inside the next phase would queue behind all
                # of this phase's stores (HWDGE rings are FIFO), stalling the
                # PE ~3us at the phase boundary
                if e + 1 < len(phases):
                    for j in range(2):
                        w_pre[(e + 1, j)] = load_w(e + 1, j)

                # ---- mm2: y2[h, m] += wd[i, h] * hidden[i, m] ----
                for hb in range(HB):
                    wd_t = wdp.tile([P, IJ, P], BF16, tag="wd", name=f"wd_{e}_{hb}")
                    di = (e * HB + hb) * P
                    nc.sync.dma_start(wd_t[:], wd[di : di + P])
                    # next phase's tokens, interleaved on the sync ring: their
                    # WAR on the x tiles cleared when this phase's mm1 ended,
                    # and the ACT ring stays stores-only (no head-of-line
                    # blocking of y2 stores behind 6MB of x transfers)
                    if e + 1 < len(phases):
                        noff, nMp = phases[e + 1]
                        for ko in range(3 * hb, min(3 * hb + 3, KO)):
                            load_x(ko, noff, nMp, nc.sync)
                    for ci, (o, sz) in enumerate(chunks):
                        po = psp.tile([P, 512], F32, tag="pu", bufs=4, name=f"po_{e}_{hb}_{ci}")
                        for j in range(IJ):
                            nc.tensor.matmul(
                                po[:, :sz],
                                wd_t[:, j, :],
                                hid[:, j, o : o + sz],
                                start=(j == 0),
                                stop=(j == IJ - 1),
                            )
                        ot = stp.tile([P, 512], F32, tag="ot", name=f"ot_{e}_{hb}_{ci}")
                        nc.vector.tensor_copy(ot[:, :sz], po[:, :sz])
                        nc.scalar.dma_start(
                            y2[hb * P : (hb + 1) * P, off + o : off + o + sz],
                            ot[:, :sz],
                        )



    _split_excess_waits(nc)
    return nc


def _route(x2d, gate_w, k):
    """Host gating in float64: top-k sets + gate values per token."""
    logits = x2d.astype(np.float64) @ gate_w.astype(np.float64).T
    logits -= logits.max(axis=-1, keepdims=True)
    p = np.exp(logits)
    p /= p.sum(axis=-1, keepdims=True)
    topk = np.argsort(-p, axis=-1, kind="stable")[:, :k]  # [S, k]
    return p, topk


def kernel(x, gate_w, w_gate_up, w_down, top_k):
    kernel.last_exec_time_ns = None
    x = np.asarray(x)
    gate_w = np.asarray(gate_w)
    w_gate_up = np.asarray(w_gate_up)
    w_down = np.asarray(w_down)
    k = int(np.asarray(top_k))

    B, S, H = x.shape
    E = gate_w.shape[0]
    I = w_down.shape[2]
    ISH = I // NCORES    # per-core I-slice (same slice position, all experts)
    IJ = ISH // P
    KO = H // P
    HB = H // P
    x2d = x.reshape(-1, H)
    n_tok = x2d.shape[0]

    p, topk = _route(x2d, gate_w, k)
    sel = [np.nonzero((topk == e).any(axis=-1))[0] for e in range(E)]
    counts = [len(s) for s in sel]
    # 8-token phase alignment: 16B-aligned bf16 x rows, 32B-aligned f32 y rows
    Mps = [max(16, -(-c // 8) * 8) for c in counts]
    offs = [0]
    for m in Mps[:-1]:
        offs.append(offs[-1] + m)
    Mtot = sum(Mps)
    phases = list(zip(offs, Mps))

    bf = ml_dtypes.bfloat16

    # token batches, phase-contiguous, zero-padded; replicated to all cores
    xt = np.zeros((H, Mtot), dtype=bf)
    for e in range(E):
        if counts[e]:
            xt[:, offs[e] : offs[e] + counts[e]] = x2d[sel[e]].T.astype(bf)

    # pre-tiled weights: every (expert, tile) is one contiguous DRAM block
    # wgu rows: part*I + c*ISH + j*P + ii ; cols: ko*P + p
    t = w_gate_up.astype(bf).reshape(E, 2, NCORES, IJ, P, KO, P)
    t = np.ascontiguousarray(t.transpose(2, 0, 1, 3, 6, 5, 4))
    wgu_t = t.reshape(NCORES, E * 2 * IJ * P, KO, P)

    t = w_down.astype(bf).reshape(E, HB, P, NCORES, IJ, P)
    t = np.ascontiguousarray(t.transpose(3, 0, 1, 5, 4, 2))
    wd_t = t.reshape(NCORES, E * HB * P, IJ, P)

    nc = _build_kernel(phases, H, ISH, E)
    trace = bool(int(os.environ.get("BASS_TRACE", "0") or "0"))

    in_maps = [
        {"xt": xt, "wgu": wgu_t[c], "wd": wd_t[c]} for c in range(NCORES)
    ]
    try:
        res = run_bass_kernel_spmd(
            nc, in_maps, core_ids=list(range(NCORES)), trace=trace
        )
    except Exception:
        # transient device/profiling hiccups: one untraced retry
        os.environ["BASS_NEVER_TRACE"] = "1"
        try:
            res = run_bass_kernel_spmd(
                nc, in_maps, core_ids=list(range(NCORES)), trace=False
            )
        finally:
            os.environ.pop("BASS_NEVER_TRACE", None)
    if res.exec_time_ns is not None:
        kernel.last_exec_time_ns = res.exec_time_ns

    # host combine: sum the 8 I-slice partials, apply gate values, scatter-add
    Y = res.results[0]["y2"].copy()
    for c in range(1, NCORES):
        Y += res.results[c]["y2"]
    y = np.zeros((n_tok, H), dtype=np.float32)
    for e in range(E):
        idx = sel[e]
        if len(idx) == 0:
            continue
        y[idx] += p[idx, e].astype(np.float32)[:, None] * Y[
            :, offs[e] : offs[e] + len(idx)
        ].T
    return y.reshape(B, S, H).astype(np.float32)


kernel.last_exec_time_ns = None


# revision 4
# speedup vs baseline: 1.0198x; 1.0132x over previous
"""MoE (top-k routing + SwiGLU expert MLP) Trainium2 kernel, 8 NeuronCores.

Strategy (v2: routing-balanced intermediate-slice sharding)
-----------------------------------------------------------
Host computes the (tiny) gating network in float64 (logits -> softmax ->
top-k sets + gate values). Every core owns the SAME I/8 slice position of
ALL four experts' weights:

    core c  ->  intermediate columns [c*1024, (c+1)*1024) of every expert

and the kernel runs 4 sequential expert phases; in phase e every core
processes expert e's full routed token batch against its I/8 weight slice.
Per-core PE work is therefore sum_e count_e (perfectly balanced regardless
of routing imbalance), instead of 2.25 * max_e count_e for the 2-cores-
per-expert layout.

Per phase (per core, bf16 matmuls, fp32 PSUM):
    mm1:  gate/up [1024(I'), M] = W(g|u)T-tiles (stationary) x xT (moving)
    swiglu: ACT silu on gate psum, DVE mul with up psum -> hidden bf16 SBUF
    mm2:  y2 [H, M] partial = WdT-tiles (stationary) x hidden (moving)

Weights are host-pre-tiled so every weight tile is ONE fully contiguous
DRAM read (786KB / 262KB); weight loads stream on the SP HWDGE ring while
x loads + y2 stores ride the ACT HWDGE ring, so input streams never queue
behind output stores. The host sums the 8 per-core partials (over the I
slices), applies the top-k gate values, and scatter-adds into [S, H].
"""

import os
import sys
import types

import ml_dtypes
import numpy as np

import concourse.bass as bass
import concourse.mybir as mybir
import concourse.tile as tile
from bass_rust import SyncInfo
from concourse.bass_utils import run_bass_kernel_spmd

NCORES = 8
P = 128
BF16 = mybir.dt.bfloat16
F32 = mybir.dt.float32


def _ensure_ntff_hook():
    """Best-effort: register the axon NTFF profile hook if the environment's
    ``antenv`` stub lacks ``axon_hooks`` (otherwise trace=True silently
    degrades and no HW exec time is reported)."""
    try:
        import antenv  # noqa: F401

        try:
            from antenv.axon_hooks import get_axon_ntff_profile_hook
        except ImportError:
            mod = types.ModuleType("antenv.axon_hooks")
            mod._HOOK = None

            def set_axon_ntff_profile_hook(hook, _m=mod):
                _m._HOOK = hook

            def get_axon_ntff_profile_hook(_m=mod):
                return _m._HOOK

            mod.set_axon_ntff_profile_hook = set_axon_ntff_profile_hook
            mod.get_axon_ntff_profile_hook = get_axon_ntff_profile_hook
            sys.modules["antenv.axon_hooks"] = mod
            import antenv as _a

            _a.axon_hooks = mod
        if get_axon_ntff_profile_hook() is None:
            from trn_agent_boot.trn_boot import _ntff_profile_via_ctypes

            from antenv.axon_hooks import set_axon_ntff_profile_hook

            hook = _ntff_profile_via_ctypes("/opt/axon/libaxon_pjrt.so")
            if hook is not None:
                set_axon_ntff_profile_hook(hook)
    except Exception:
        pass


_ensure_ntff_hook()


def _split_excess_waits(nc, max_sync=1):
    """walrus in this container rejects >~2 sync commands per instruction
    (CoreV3 setupSyncWait).  Hoist excess sem waits onto NoOps that run
    immediately before the offending instruction on the same engine."""
    for bb in nc.m.functions[0].blocks:
        new, changed = [], False
        for ins in bb.instructions:
            si = ins.sync_info
            if si is None:
                new.append(ins)
                continue
            waits = list(si.on_wait)
            n_upd = len(si.on_update)
            if len(waits) + n_upd > max_sync and len(waits) > 1:
                keep = max(1, max_sync - n_upd)
                extra, kept = waits[: len(waits) - keep], waits[len(waits) - keep :]
                for j in range(0, len(extra), max_sync):
                    nop = mybir.InstNoOp(name=f"{ins.name}_waitsplit_{j}")
                    nop.engine = ins.engine
                    nop.sync_info = SyncInfo(
                        on_wait=extra[j : j + max_sync], on_update=[]
                    )
                    nc.register_instruction(nop)
                    new.append(nop)
                ins.sync_info = SyncInfo(on_wait=kept, on_update=si.on_update)
                changed = True
            new.append(ins)
        if changed:
            bb.instructions = new


def _chunks_of(Mp):
    """Balanced output-column chunks <=512 (PSUM bank is 512 f32), 16-aligned.

    Balanced (not [512..., remainder]) is load-bearing: with ldw-opt disabled
    every matmul re-issues a ~107ns LDWEIGHTS, which only stays hidden when
    the preceding matmul streams >~128 columns — a tiny remainder chunk would
    stall the PE on every (j, k) pass."""
    n_ch = -(-Mp // 512)
    base = (Mp // n_ch) // 16 * 16
    sizes = [base] * n_ch
    for i in range(-(-(Mp - base * n_ch) // 16)):
        sizes[i] += 16
    sizes[-1] = Mp - sum(sizes[:-1])
    out, o = [], 0
    for s in sizes:
        out.append((o, s))
        o += s
    return out


def _build_kernel(phases, H, ISH, E):
    """One-core program (SPMD x8): 4 expert phases of a SwiGLU MLP shard.

    Inputs : xt  [H, Mtot] bf16           (token batches, phase-contiguous)
             wgu [E*2*IJ*P, KO, P] bf16   (pre-tiled gate/up weight tiles)
             wd  [E*HB*P, IJ, P] bf16     (pre-tiled down-proj weight tiles)
    Output : y2  [H, Mtot] fp32           (partial over this core's I-slice)
    """
    KO = H // P          # contraction tiles of mm1 (hidden dim)
    IJ = ISH // P        # i-tiles of this core's intermediate slice
    HB = H // P          # output-row tiles of mm2
    Mtot = sum(Mp for _, Mp in phases)
    Mmax = max(Mp for _, Mp in phases)

    nc = bass.Bass("TRN2", num_devices=NCORES)
    xt = nc.dram_tensor("xt", [H, Mtot], BF16, kind="ExternalInput")
    wgu = nc.dram_tensor("wgu", [E * 2 * IJ * P, KO, P], BF16, kind="ExternalInput")
    wd = nc.dram_tensor("wd", [E * HB * P, IJ, P], BF16, kind="ExternalInput")
    y2 = nc.dram_tensor("y2", [H, Mtot], BF16, kind="ExternalOutput")

    with tile.TileContext(nc) as tc:
        with (
            tc.tile_pool(name="xp", bufs=1) as xp,
            tc.tile_pool(name="hp", bufs=1) as hp,
            tc.tile_pool(name="wp", bufs=2) as wp,
            tc.tile_pool(name="wdp", bufs=8) as wdp,
            tc.tile_pool(name="sgp", bufs=3) as sgp,
            tc.tile_pool(name="stp", bufs=6) as stp,
            tc.tile_pool(name="psp", bufs=4, space="PSUM") as psp,
        ):
            # x k-tiles: one tile per ko — tile dependencies are count-based
            # per tile, so separate tiles let each matmul wait only on the
            # k-slab it reads (progressive consumption during the ramp)
            x_sb = [
                xp.tile([P, Mmax], BF16, tag="x", bufs=KO, name=f"x{ko}")
                for ko in range(KO)
            ]
            hid = hp.tile([P, IJ, Mmax], BF16)

            def load_x(ko, off, Mp, engine):
                engine.dma_start(
                    x_sb[ko][:, :Mp], xt[ko * P : (ko + 1) * P, off : off + Mp]
                )

            def load_w(e, j):
                # wg on the SP ring, wu on the ACT ring: per-ring bandwidth
                # (~170-200GB/s) is the scarce resource during the ramp
                wg_t = wp.tile([P, KO, P], BF16, tag="wg", name=f"wg_{e}_{j}")
                gi = ((e * 2 + 0) * IJ + j) * P
                nc.sync.dma_start(wg_t[:], wgu[gi : gi + P])
                wu_t = wp.tile([P, KO, P], BF16, tag="wu", name=f"wu_{e}_{j}")
                ui = ((e * 2 + 1) * IJ + j) * P
                nc.scalar.dma_start(wu_t[:], wgu[ui : ui + P])
                return wg_t, wu_t

            # phase-0 j0/j1 weights BEFORE the x burst (rings are FIFO: the
            # first matmul must not queue behind ~3MB of x per ring), then
            # phase-0 tokens alternating across both HWDGE rings: a single
            # ring sustains well under the ~300GB/s the first j-pass consumes
            w_pre = {(0, 0): load_w(0, 0)}
            for ko in range(KO):
                load_x(ko, phases[0][0], phases[0][1], nc.scalar if ko % 2 == 0 else nc.sync)
                if ko == 5:
                    w_pre[(0, 1)] = load_w(0, 1)

            for e, (off, Mp) in enumerate(phases):
                chunks = _chunks_of(Mp)
                # ---- mm1 + SwiGLU ----
                for j in range(IJ):
                    wg_t, wu_t = w_pre.pop((e, j), None) or load_w(e, j)
                    pgs = [
                        psp.tile([P, 512], F32, tag="pg", bufs=4, name=f"pg_{e}_{j}_{ci}")
                        for ci in range(len(chunks))
                    ]
                    pus = [
                        psp.tile([P, 512], F32, tag="pu", bufs=4, name=f"pu_{e}_{j}_{ci}")
                        for ci in range(len(chunks))
                    ]
                    # k-outer: each stationary loaded once per (j, k); x tiles
                    # consumed in DMA arrival order (phase-0 ramp overlap)
                    for k in range(KO):
                        for ci, (o, sz) in enumerate(chunks):
                            nc.tensor.matmul(
                                pgs[ci][:, :sz],
                                wg_t[:, k, :],
                                x_sb[k][:, o : o + sz],
                                start=(k == 0),
                                stop=(k == KO - 1),
                            )
                        for ci, (o, sz) in enumerate(chunks):
                            nc.tensor.matmul(
                                pus[ci][:, :sz],
                                wu_t[:, k, :],
                                x_sb[k][:, o : o + sz],
                                start=(k == 0),
                                stop=(k == KO - 1),
                            )
                    for ci, (o, sz) in enumerate(chunks):
                        sg = sgp.tile([P, 512], F32, tag="sg", name=f"sg_{e}_{j}_{ci}")
                        nc.scalar.activation(
                            sg[:, :sz],
                            pgs[ci][:, :sz],
                            mybir.ActivationFunctionType.Silu,
                        )
                        nc.vector.tensor_mul(
                            hid[:, j, o : o + sz], sg[:, :sz], pus[ci][:, :sz]
                        )

                # hoist the next phase's first weight pairs to BEFORE any y2
                # stores are issued: the wu stream rides the ACT ring, and a
                # wu DMA issued inside the next phase would queue behind all
                # of this phase's stores (HWDGE rings are FIFO), stalling the
                # PE ~3us at the phase boundary
                if e + 1 < len(phases):
                    for j in range(2):
                        w_pre[(e + 1, j)] = load_w(e + 1, j)

                # ---- mm2: y2[h, m] += wd[i, h] * hidden[i, m] ----
                for hb in range(HB):
                    wd_t = wdp.tile([P, IJ, P], BF16, tag="wd", name=f"wd_{e}_{hb}")
                    di = (e * HB + hb) * P
                    nc.sync.dma_start(wd_t[:], wd[di : di + P])
                    # next phase's tokens, interleaved on the sync ring: their
                    # WAR on the x tiles cleared when this phase's mm1 ended,
                    # and the ACT ring stays stores-only (no head-of-line
                    # blocking of y2 stores behind 6MB of x transfers)
                    if e + 1 < len(phases):
                        noff, nMp = phases[e + 1]
                        for ko in range(3 * hb, min(3 * hb + 3, KO)):
                            load_x(ko, noff, nMp, nc.sync if ko % 2 else nc.scalar)
                    for ci, (o, sz) in enumerate(chunks):
                        po = psp.tile([P, 512], F32, tag="pu", bufs=4, name=f"po_{e}_{hb}_{ci}")
                        for j in range(IJ):
                            nc.tensor.matmul(
                                po[:, :sz],
                                wd_t[:, j, :],
                                hid[:, j, o : o + sz],
                                start=(j == 0),
                                stop=(j == IJ - 1),
                            )
                        ot = stp.tile([P, 512], BF16, tag="ot", name=f"ot_{e}_{hb}_{ci}")
                        nc.vector.tensor_copy(ot[:, :sz], po[:, :sz])
                        nc.scalar.dma_start(
                            y2[hb * P : (hb + 1) * P, off + o : off + o + sz],
                            ot[:, :sz],
                        )



    _split_excess_waits(nc)
    return nc


def _route(x2d, gate_w, k):
    """Host gating in float64: top-k sets + gate values per token."""
    logits = x2d.astype(np.float64) @ gate_w.astype(np.float64).T
    logits -= logits.max(axis=-1, keepdims=True)
    p = np.exp(logits)
    p /= p.sum(axis=-1, keepdims=True)
    topk = np.argsort(-p, axis=-1, kind="stable")[:, :k]  # [S, k]
    return p, topk


def kernel(x, gate_w, w_gate_up, w_down, top_k):
    kernel.last_exec_time_ns = None
    x = np.asarray(x)
    gate_w = np.asarray(gate_w)
    w_gate_up = np.asarray(w_gate_up)
    w_down = np.asarray(w_down)
    k = int(np.asarray(top_k))

    B, S, H = x.shape
    E = gate_w.shape[0]
    I = w_down.shape[2]
    ISH = I // NCORES    # per-core I-slice (same slice position, all experts)
    IJ = ISH // P
    KO = H // P
    HB = H // P
    x2d = x.reshape(-1, H)
    n_tok = x2d.shape[0]

    p, topk = _route(x2d, gate_w, k)
    sel = [np.nonzero((topk == e).any(axis=-1))[0] for e in range(E)]
    counts = [len(s) for s in sel]
    # 8-token phase alignment: 16B-aligned bf16 x rows, 32B-aligned f32 y rows
    Mps = [max(16, -(-c // 8) * 8) for c in counts]
    offs = [0]
    for m in Mps[:-1]:
        offs.append(offs[-1] + m)
    Mtot = sum(Mps)
    phases = list(zip(offs, Mps))

    bf = ml_dtypes.bfloat16

    # token batches, phase-contiguous, zero-padded; replicated to all cores
    xt = np.zeros((H, Mtot), dtype=bf)
    for e in range(E):
        if counts[e]:
            xt[:, offs[e] : offs[e] + counts[e]] = x2d[sel[e]].T.astype(bf)

    # pre-tiled weights: every (expert, tile) is one contiguous DRAM block
    # wgu rows: part*I + c*ISH + j*P + ii ; cols: ko*P + p
    t = w_gate_up.astype(bf).reshape(E, 2, NCORES, IJ, P, KO, P)
    t = np.ascontiguousarray(t.transpose(2, 0, 1, 3, 6, 5, 4))
    wgu_t = t.reshape(NCORES, E * 2 * IJ * P, KO, P)

    t = w_down.astype(bf).reshape(E, HB, P, NCORES, IJ, P)
    t = np.ascontiguousarray(t.transpose(3, 0, 1, 5, 4, 2))
    wd_t = t.reshape(NCORES, E * HB * P, IJ, P)

    nc = _build_kernel(phases, H, ISH, E)
    trace = bool(int(os.environ.get("BASS_TRACE", "0") or "0"))

    in_maps = [
        {"xt": xt, "wgu": wgu_t[c], "wd": wd_t[c]} for c in range(NCORES)
    ]
    try:
        res = run_bass_kernel_spmd(
            nc, in_maps, core_ids=list(range(NCORES)), trace=trace
        )
    except Exception:
        # transient device/profiling hiccups: one untraced retry
        os.environ["BASS_NEVER_TRACE"] = "1"
        try:
            res = run_bass_kernel_spmd(
                nc, in_maps, core_ids=list(range(NCORES)), trace=False
            )
        finally:
            os.environ.pop("BASS_NEVER_TRACE", None)
    if res.exec_time_ns is not None:
        kernel.last_exec_time_ns = res.exec_time_ns

    # host combine: sum the 8 I-slice partials, apply gate values, scatter-add
    Y = res.results[0]["y2"].astype(np.float32)
    for c in range(1, NCORES):
        Y += res.results[c]["y2"].astype(np.float32)
    y = np.zeros((n_tok, H), dtype=np.float32)
    for e in range(E):
        idx = sel[e]
        if len(idx) == 0:
            continue
        y[idx] += p[idx, e].astype(np.float32)[:, None] * Y[
            :, offs[e] : offs[e] + len(idx)
        ].T
    return y.reshape(B, S, H).astype(np.float32)


kernel.last_exec_time_ns = None
